# revision 16
# baseline (speedup 1.0000x reference)
"""Sigmoid-attention (DiffAttention) kernel for 8 Trainium2 NeuronCores.

Problem:  N=L=4096, H=8 heads, M=D=64.
    scores[n,l,h] = sigmoid(q[n,h,:] . k[l,h,:])
    out[n,h,:]    = (scores @ v) / sum_l(scores)        (per head)

Sharding: one head per core (8 heads == 8 cores). Each core gets its
head's Q/K transposed to [64, 4096] (duplicated onto both SBUF
partition halves) plus V packed as [V | ones] tiles, computes the full
attention for that head, and returns the head output transposed
([64, 4096]); the host restores [4096, 8, 64].

Per-core dataflow (fp16 matmuls, fp32 PSUM accumulation), with the PE
array row-tiled 64x128 (tiles T0 = SBUF partitions 0-63 and T8 =
64-127) so two contraction-64 matmuls stream concurrently:
    S^T[l,n]   = matmul(lhsT=K^T[:,l_tile], rhs=Q^T[:,n_chunk])
                 even l_tiles on T0, odd on T8          (PE, 2 streams)
    A^T[l,n]   = sigmoid(S^T)  fp32 PSUM -> fp16 SBUF   (ACT)
    acc_a     += matmul(lhsT=[V|1][l 0:64],   rhs=A^T[0:64])    (T0)
    acc_b     += matmul(lhsT=[V|1][l 64:128], rhs=A^T[64:128])  (T8)
    out        = (acc_a+acc_b)[0:64] / (acc_a+acc_b)[64]  (DVE+GpSimd)
"""

from contextlib import ExitStack

import numpy as np

import concourse.bass as bass
import concourse.mybir as mybir
import concourse.tile as tile
from concourse import bacc
from concourse.bass import ts
from concourse.bass_utils import run_bass_kernel_spmd

N, L, H, M, D = 4096, 4096, 8, 64, 64
NCORES = 8
NCHUNK = 1024  # n columns per PSUM chunk
NCHUNKS = N // NCHUNK
LTILES = L // 128
VW = D + 1  # V columns + ones column
SKEW = 6  # mm2 trails mm1 by SKEW l_tiles (even) so PE always has ready work
CDT = mybir.dt.float16  # PE input dtype
FP32 = mybir.dt.float32
SIGMOID = mybir.ActivationFunctionType.Sigmoid

_CACHE: dict = {}


def build_nc():
    nc = bacc.Bacc("TRN2", target_bir_lowering=False, debug=False)

    q2_d = nc.dram_tensor("q2", [128, N], CDT, kind="ExternalInput").ap()
    k2_d = nc.dram_tensor("k2", [128, L], CDT, kind="ExternalInput").ap()
    v1_d = nc.dram_tensor("v1", [128, LTILES * VW], CDT, kind="ExternalInput").ap()
    out_d = nc.dram_tensor("out", [D, N], FP32, kind="ExternalOutput").ap()

    with ExitStack() as ctx:
        tc = ctx.enter_context(tile.TileContext(nc))
        const = ctx.enter_context(tc.tile_pool(name="const", bufs=1))
        apool = ctx.enter_context(tc.tile_pool(name="apool", bufs=SKEW + 2))
        io = ctx.enter_context(tc.tile_pool(name="io", bufs=2))
        psA = ctx.enter_context(tc.tile_pool(name="psA", bufs=2, space="PSUM"))
        psAcc = ctx.enter_context(tc.tile_pool(name="psAcc", bufs=1, space="PSUM"))

        # Split the input loads so the first l_tiles / n-chunks unblock early.
        q2_s = const.tile([128, N], CDT)
        k2_s = const.tile([128, L], CDT)
        v1_s = const.tile([128, LTILES * VW], CDT)
        # First pieces extra-fine so the first mm1 pair unblocks ASAP.
        nc.sync.dma_start(out=k2_s[:, 0:256], in_=k2_d[:, 0:256])
        nc.sync.dma_start(out=q2_s[:, 0:512], in_=q2_d[:, 0:512])
        nc.sync.dma_start(out=q2_s[:, 512:1024], in_=q2_d[:, 512:1024])
        nc.sync.dma_start(out=k2_s[:, 256:1024], in_=k2_d[:, 256:1024])
        for ci in range(NCHUNKS):
            cs = ci * NCHUNK
            if ci > 0:
                nc.sync.dma_start(
                    out=k2_s[:, cs : cs + NCHUNK], in_=k2_d[:, cs : cs + NCHUNK]
                )
            nc.sync.dma_start(
                out=v1_s[:, ci * 8 * VW : (ci + 1) * 8 * VW],
                in_=v1_d[:, ci * 8 * VW : (ci + 1) * 8 * VW],
            )
            if ci > 0:
                nc.sync.dma_start(
                    out=q2_s[:, cs : cs + NCHUNK], in_=q2_d[:, cs : cs + NCHUNK]
                )

        def mm1pair(ci, lt, sT_e, sT_o):
            # even l_tile on T0 (partitions 0-63), odd on T8 (64-127);
            # interleave halves so the two tiles stream concurrently.
            cs = ci * NCHUNK
            ke = k2_s[0:64, ts(lt, 128)]
            ko = k2_s[64:128, ts(lt + 1, 128)]
            for h in range(NCHUNK // 512):
                qsl = slice(cs + h * 512, cs + (h + 1) * 512)
                nc.tensor.matmul(
                    sT_e[:, ts(h, 512)], ke, q2_s[0:64, qsl], start=True, stop=True
                )
                nc.tensor.matmul(
                    sT_o[:, ts(h, 512)], ko, q2_s[64:128, qsl], start=True, stop=True
                )

        def sig(sT, aT):
            nc.scalar.activation(aT, sT, SIGMOID)

        def mm2(lt, aT, acc_a, acc_b):
            va = v1_s[0:64, lt * VW : (lt + 1) * VW]
            vb = v1_s[64:128, lt * VW : (lt + 1) * VW]
            first, last = lt == 0, lt == LTILES - 1
            for h in range(NCHUNK // 512):
                hs = ts(h, 512)
                nc.tensor.matmul(acc_a[:, hs], va, aT[0:64, hs], start=first, stop=last)
                nc.tensor.matmul(acc_b[:, hs], vb, aT[64:128, hs], start=first, stop=last)

        def epilogue_half(ci, h, acc_a, acc_b):
            # out[:, 512-half] = sum[0:D] / sum[D] (normalizer row), where
            # sum = acc_a + acc_b. Runs on DVE/GpSimd; PE is not involved.
            cs = ci * NCHUNK + h * 512
            hs = ts(h, 512)
            tmp = io.tile([VW, 512], FP32, tag="tmp", name="tmp")
            nc.vector.tensor_copy(tmp, acc_a[:, hs])
            summ = io.tile([VW, 512], FP32, tag="summ", name="summ")
            nc.vector.tensor_add(summ, tmp, acc_b[:, hs])
            norm_sb = io.tile([1, 512], FP32, tag="norm", name="norm")
            nc.vector.tensor_copy(norm_sb, summ[D : D + 1, :])
            bc = io.tile([D, 512], FP32, tag="bc", name="bc")
            nc.gpsimd.partition_broadcast(bc, norm_sb, channels=D)
            rec = io.tile([D, 512], FP32, tag="rec", name="rec")
            nc.vector.reciprocal_approx_fast(out=rec, in_=bc)
            o = io.tile([D, 512], FP32, tag="o", name="o")
            nc.vector.tensor_mul(o, summ[0:D, :], rec)
            nc.sync.dma_start(out=out_d[:, cs : cs + 512], in_=o)

        for ci in range(NCHUNKS):
            cs = ci * NCHUNK
            acc_a = psAcc.tile([VW, NCHUNK], FP32, tag="acc_a")
            acc_b = psAcc.tile([VW, NCHUNK], FP32, tag="acc_b")
            aTs = [None] * LTILES

            def mm1sig(ci, lt):
                sT_e = psA.tile([128, NCHUNK], FP32, tag="sT", name="sT")
                sT_o = psA.tile([128, NCHUNK], FP32, tag="sT", name="sT")
                mm1pair(ci, lt, sT_e, sT_o)
                aTs[lt] = apool.tile([128, NCHUNK], CDT, tag="aT", name="aT")
                aTs[lt + 1] = apool.tile([128, NCHUNK], CDT, tag="aT", name="aT")
                sig(sT_e, aTs[lt])
                sig(sT_o, aTs[lt + 1])

            for lt in range(0, SKEW, 2):
                mm1sig(ci, lt)
            for lt in range(SKEW, LTILES, 2):
                # Sandwich the mm1 pair inside the trailing mm2 batch: the
                # first mm2 keeps PE busy while mm1 waits for its sT slot,
                # and mm1 lands early enough that ACT never starves.
                mm2(lt - SKEW, aTs[lt - SKEW], acc_a, acc_b)
                mm1sig(ci, lt)
                mm2(lt - SKEW + 1, aTs[lt - SKEW + 1], acc_a, acc_b)
                aTs[lt - SKEW] = aTs[lt - SKEW + 1] = None
            # Drain: emit all h0 halves, then the h0 epilogue (DVE/GpSimd)
            # overlapped with the h1 halves, then the h1 epilogue.
            drain = range(LTILES - SKEW, LTILES)
            for h in range(2):
                for lt in drain:
                    va = v1_s[0:64, lt * VW : (lt + 1) * VW]
                    vb = v1_s[64:128, lt * VW : (lt + 1) * VW]
                    first, last = lt == 0, lt == LTILES - 1
                    hs = ts(h, 512)
                    nc.tensor.matmul(
                        acc_a[:, hs], va, aTs[lt][0:64, hs], start=first, stop=last
                    )
                    nc.tensor.matmul(
                        acc_b[:, hs], vb, aTs[lt][64:128, hs], start=first, stop=last
                    )
                epilogue_half(ci, h, acc_a, acc_b)
            for lt in drain:
                aTs[lt] = None

    nc.compile()
    return nc


def get_nc():
    if "nc" not in _CACHE:
        _CACHE["nc"] = build_nc()
    return _CACHE["nc"]


def make_in_maps(queries, keys, values):
    np_cdt = mybir.dt.np(CDT)
    in_maps = []
    for h in range(NCORES):
        qT = np.ascontiguousarray(queries[:, h, :].T.astype(np_cdt))
        kT = np.ascontiguousarray(keys[:, h, :].T.astype(np_cdt))
        v1 = np.empty((L, VW), np_cdt)
        v1[:, :D] = values[:, h, :]
        v1[:, D] = 1.0
        v1p = np.ascontiguousarray(
            v1.reshape(LTILES, 128, VW).transpose(1, 0, 2).reshape(128, LTILES * VW)
        )
        in_maps.append(
            {
                "q2": np.vstack([qT, qT]),
                "k2": np.vstack([kT, kT]),
                "v1": v1p,
            }
        )
    return in_maps


def run(queries, keys, values, trace=False):
    """Returns (out [N,H,D] fp32, BassKernelResults)."""
    nc = get_nc()
    in_maps = make_in_maps(queries, keys, values)
    res = run_bass_kernel_spmd(nc, in_maps, core_ids=list(range(NCORES)), trace=trace)
    out = np.empty((N, H, D), np.float32)
    for h in range(NCORES):
        out[:, h, :] = res.results[h]["out"].T
    return out, res


def kernel(queries, keys, values):
    out, _ = run(np.asarray(queries), np.asarray(keys), np.asarray(values))
    return out
